# revision 1
# baseline (speedup 1.0000x reference)
# Trainium2 Bass kernel for the MEGNet edge model:
#   out = relu(concat([src, dest, edge_attr, u[batch]], 1) @ W1 + b1) @ W2 + b2
#
# Strategy (8 NeuronCores, SPMD, edges sharded contiguously):
#  * All tensors are shipped to the device in a transposed, feature-major
#    layout [128, E_pad] so the PE array can contract over features without
#    any on-chip transposes; the host transposes shards (layout choice made
#    while sharding) and transposes the output back.
#  * comb @ W1 decomposes into src@W1a + dest@W1b + edge_attr@W1c +
#    u[batch]@W1d.  The u[batch] term plus b1 is folded into a per-group
#    table z = u @ W1d + b1 [G, 128]; since batch is sorted, each 512-edge
#    tile only spans a few consecutive groups, so z[batch] is applied with
#    one extra small matmul per tile: lhsT = the k_s candidate z-rows of
#    that tile, rhs = a one-hot selection matrix built on the host.
#  * PSUM accumulates the 4 matmul terms; ScalarE applies ReLU (PSUM->SBUF);
#    the second matmul uses W2; VectorE adds b2 (per-partition vector).
import os
import numpy as np

N_CORES = 8
P = 128      # feature dim == SBUF partitions
TILE = 512   # edges per matmul tile (one PSUM bank of fp32)
CH = 7       # matmul tiles per DMA chunk (98 tiles = 14 chunks of 7)

# Matmul operand dtype: "f32" (exact, 2 HW passes), "f32r" (same fp32 bits,
# single-pass reduced-precision multiply), "bf16" (half DMA traffic too).
MM_DTYPE = os.environ.get("KERNEL_MM_DTYPE", "f32r")

_prog_cache = {}


def _np_mm_dtype():
    if MM_DTYPE == "bf16":
        import ml_dtypes
        return ml_dtypes.bfloat16
    return np.float32


def _build_program(T, k_s):
    import concourse.bacc as bacc
    import concourse.tile as tile
    from concourse import mybir

    f32 = mybir.dt.float32
    mdt = {"f32": mybir.dt.float32, "f32r": mybir.dt.float32r,
           "bf16": mybir.dt.bfloat16}[MM_DTYPE]
    Relu = mybir.ActivationFunctionType.Relu
    Epad = T * TILE

    nc = bacc.Bacc("TRN2", target_bir_lowering=False, debug=False,
                   num_devices=N_CORES)
    srcT = nc.dram_tensor("srcT", [P, Epad], mdt, kind="ExternalInput")
    destT = nc.dram_tensor("destT", [P, Epad], mdt, kind="ExternalInput")
    eaT = nc.dram_tensor("eaT", [P, Epad], mdt, kind="ExternalInput")
    w1d = nc.dram_tensor("w1", [3 * P, P], mdt, kind="ExternalInput")
    w2d = nc.dram_tensor("w2", [P, P], mdt, kind="ExternalInput")
    b2d = nc.dram_tensor("b2c", [P, 1], f32, kind="ExternalInput")
    seld = nc.dram_tensor("sel", [k_s, Epad], mdt, kind="ExternalInput")
    zwd = nc.dram_tensor("zw", [k_s, T * P], mdt, kind="ExternalInput")
    outT = nc.dram_tensor("outT", [P, Epad], f32, kind="ExternalOutput")

    assert T % CH == 0
    n_chunks = T // CH
    CW = CH * TILE  # chunk width in edges

    with tile.TileContext(nc) as tc:
        with (
            tc.tile_pool(name="const", bufs=1) as constp,
            tc.tile_pool(name="inp", bufs=2) as inp,
            tc.tile_pool(name="hp", bufs=4) as hp,
            tc.tile_pool(name="outp", bufs=2) as outp,
            tc.tile_pool(name="psum", bufs=3, space="PSUM") as psum,
        ):
            w1a = constp.tile([P, P], mdt, tag="w1a", name="w1a")
            w1b = constp.tile([P, P], mdt, tag="w1b", name="w1b")
            w1c = constp.tile([P, P], mdt, tag="w1c", name="w1c")
            w2s = constp.tile([P, P], mdt, tag="w2s", name="w2s")
            b2s = constp.tile([P, 1], f32, tag="b2s", name="b2s")
            zws = constp.tile([k_s, T * P], mdt, tag="zws", name="zws")
            nc.gpsimd.dma_start(zws[:], zwd[:])
            nc.sync.dma_start(w1a[:], w1d[0:P, :])
            nc.sync.dma_start(w1b[:], w1d[P:2 * P, :])
            nc.sync.dma_start(w1c[:], w1d[2 * P:3 * P, :])
            nc.sync.dma_start(w2s[:], w2d[:])
            nc.sync.dma_start(b2s[:], b2d[:])

            for c in range(n_chunks):
                base = c * CW
                st = inp.tile([P, CW], mdt, tag="src", name=f"st{c}")
                nc.sync.dma_start(st[:], srcT[:, base:base + CW])
                dt = inp.tile([P, CW], mdt, tag="dest", name=f"dt{c}")
                nc.gpsimd.dma_start(dt[:], destT[:, base:base + CW])
                et = inp.tile([P, CW], mdt, tag="ea", name=f"et{c}")
                nc.sync.dma_start(et[:], eaT[:, base:base + CW])
                slt = inp.tile([k_s, CW], mdt, tag="sel", name=f"slt{c}")
                nc.gpsimd.dma_start(slt[:], seld[:, base:base + CW])
                ot = outp.tile([P, CW], f32, tag="o", name=f"ot{c}")

                for tl in range(CH):
                    t = c * CH + tl
                    cs = slice(tl * TILE, (tl + 1) * TILE)
                    p1 = psum.tile([P, TILE], f32, tag="p1", name=f"p1_{t}")
                    nc.tensor.matmul(p1[:], w1a[:], st[:, cs],
                                     start=True, stop=False)
                    nc.tensor.matmul(p1[:], w1b[:], dt[:, cs],
                                     start=False, stop=False)
                    nc.tensor.matmul(p1[:], w1c[:], et[:, cs],
                                     start=False, stop=False)
                    for j0 in range(0, k_s, P):
                        j1 = min(j0 + P, k_s)
                        nc.tensor.matmul(p1[:],
                                         zws[j0:j1, t * P:(t + 1) * P],
                                         slt[j0:j1, cs],
                                         start=False, stop=(j1 == k_s))
                    h = hp.tile([P, TILE], mdt, tag="h", name=f"h{t}")
                    nc.scalar.activation(h[:], p1[:], Relu)
                    p2 = psum.tile([P, TILE], f32, tag="p2", name=f"p2_{t}")
                    nc.tensor.matmul(p2[:], w2s[:], h[:], start=True, stop=True)
                    nc.vector.tensor_scalar_add(ot[:, cs], p2[:], b2s[:])

                if c % 2 == 0:
                    nc.sync.dma_start(outT[:, base:base + CW], ot[:])
                else:
                    nc.gpsimd.dma_start(outT[:, base:base + CW], ot[:])

    nc.compile()
    return nc


def _get_program(T, k_s):
    key = (T, k_s)
    if key not in _prog_cache:
        _prog_cache[key] = _build_program(T, k_s)
    return _prog_cache[key]


def _install_profile_shim():
    """Optional: enable NTFF profiling under axon (KERNEL_PROFILE=1)."""
    import sys, types
    if "antenv.axon_hooks" not in sys.modules:
        mod = types.ModuleType("antenv.axon_hooks")
        mod._hook = None
        mod.set_axon_ntff_profile_hook = lambda h: setattr(mod, "_hook", h)
        mod.get_axon_ntff_profile_hook = lambda: mod._hook
        sys.modules["antenv.axon_hooks"] = mod
        try:
            import antenv
            antenv.axon_hooks = mod
        except ImportError:
            pass
        try:
            from trn_agent_boot.trn_boot import _ntff_profile_via_ctypes
            mod.set_axon_ntff_profile_hook(
                _ntff_profile_via_ctypes("/opt/axon/libaxon_pjrt.so"))
        except Exception:
            pass
    import concourse.bass_utils as bass_utils
    bass_utils.upload_artifacts = lambda tmpdir: tmpdir


def kernel(src, dest, edge_attr, u, batch, W1, b1, W2, b2):
    src = np.asarray(src, dtype=np.float32)
    dest = np.asarray(dest, dtype=np.float32)
    edge_attr = np.asarray(edge_attr, dtype=np.float32)
    u = np.asarray(u, dtype=np.float32)
    W1 = np.asarray(W1, dtype=np.float32)
    b1 = np.asarray(b1, dtype=np.float32)
    W2 = np.asarray(W2, dtype=np.float32)
    b2 = np.asarray(b2, dtype=np.float32)
    b = np.asarray(batch).astype(np.int64)

    E, D = src.shape
    G = u.shape[0]
    assert D == P and E % N_CORES == 0
    E0 = E // N_CORES
    CW = CH * TILE
    Epad = ((E0 + CW - 1) // CW) * CW
    T = Epad // TILE

    # Fold u[batch] @ W1d + b1 into a per-group table (tiny: G x D).
    z = (u @ W1[3 * D:4 * D] + b1).astype(np.float32)  # [G, D]

    # Per-core: tile-local group offsets for the z-selection matmul.
    g0s, js = [], []
    k_s = 1
    for c in range(N_CORES):
        bc = b[c * E0:(c + 1) * E0]
        bp = np.concatenate([bc, np.full(Epad - E0, bc[-1], dtype=np.int64)])
        per_tile = bp.reshape(T, TILE)
        g0 = per_tile.min(axis=1)                 # [T]
        j = bp - np.repeat(g0, TILE)              # [Epad], >= 0
        g0s.append(g0)
        js.append(j)
        k_s = max(k_s, int(j.max()) + 1)

    mmdt = _np_mm_dtype()
    in_maps = []
    w1_in = np.ascontiguousarray(W1[:3 * D]).astype(mmdt)
    w2_in = np.ascontiguousarray(W2).astype(mmdt)
    b2_in = np.ascontiguousarray(b2.reshape(P, 1))
    for c in range(N_CORES):
        sl = slice(c * E0, (c + 1) * E0)

        def tr(x):
            out = np.zeros((P, Epad), dtype=mmdt)
            out[:, :E0] = x[sl].T.astype(mmdt)
            return out

        selc = np.zeros((k_s, Epad), dtype=mmdt)
        selc[js[c], np.arange(Epad)] = 1.0
        selc[:, E0:] = 0.0  # pad edges contribute nothing
        gidx = np.clip(g0s[c][:, None] + np.arange(k_s)[None, :], 0, G - 1)
        zwc = np.ascontiguousarray(
            z[gidx].transpose(1, 0, 2).reshape(k_s, T * P)).astype(mmdt)
        in_maps.append({
            "srcT": tr(src), "destT": tr(dest), "eaT": tr(edge_attr),
            "w1": w1_in, "w2": w2_in, "b2c": b2_in,
            "sel": selc, "zw": zwc,
        })

    profile = os.environ.get("KERNEL_PROFILE", "") == "1"
    if profile:
        _install_profile_shim()

    nc = _get_program(T, k_s)
    from concourse.bass_utils import run_bass_kernel_spmd
    kwargs = {}
    if profile:
        kwargs["trace"] = True
        if os.environ.get("KERNEL_PROFILE_ALL", "") == "1":
            kwargs["trace_cores"] = list(range(N_CORES))
    res = run_bass_kernel_spmd(nc, in_maps, core_ids=list(range(N_CORES)),
                               **kwargs)
    if profile and res.exec_time_ns is not None:
        with open("/tmp/kernel_exec_ns.txt", "w") as f:
            f.write(str(res.exec_time_ns))
        print(f"HW exec time: {res.exec_time_ns} ns")

    out = np.empty((E, P), dtype=np.float32)
    for c in range(N_CORES):
        out[c * E0:(c + 1) * E0] = res.results[c]["outT"][:, :E0].T
    return out



# revision 2
# speedup vs baseline: 1.2009x; 1.2009x over previous
# Trainium2 Bass kernel for the MEGNet edge model:
#   out = relu(concat([src, dest, edge_attr, u[batch]], 1) @ W1 + b1) @ W2 + b2
#
# Strategy (8 NeuronCores, SPMD, edges sharded contiguously):
#  * All tensors are shipped to the device in a transposed, feature-major
#    layout [128, E_pad] so the PE array can contract over features without
#    any on-chip transposes; the host transposes shards (layout choice made
#    while sharding) and transposes the output back.
#  * The three big input streams are sent in bf16 (the correctness budget
#    easily allows it: measured rel err ~3e-3 vs a 2e-2 gate) and the output
#    is also stored/DMAed as bf16 and upcast to fp32 on the host.  This
#    halves HBM traffic vs fp32 -- the kernel is memory-bound.
#  * The streams are interleaved chunk-wise in DRAM as [src|dest|ea] blocks
#    so each chunk needs ONE big contiguous DMA (~5.5 MB) instead of three.
#  * comb @ W1 decomposes into src@W1a + dest@W1b + edge_attr@W1c +
#    u[batch]@W1d.  The u[batch] term plus b1 is folded into a per-group
#    table z = u @ W1d + b1 [G, 128]; since batch is sorted, each 512-edge
#    tile only spans a few consecutive groups, so z[batch] is applied with
#    one extra small matmul per tile: lhsT = the k_s candidate z-rows of
#    that tile, rhs = a one-hot selection matrix built on the host.
#  * PSUM accumulates the 4 matmul terms; ScalarE applies ReLU (PSUM->SBUF);
#    the second matmul uses W2; VectorE adds b2 (per-partition vector).
import os
import numpy as np

N_CORES = 8
P = 128      # feature dim == SBUF partitions
TILE = 512   # edges per matmul tile (one PSUM bank of fp32)
CH = 14      # matmul tiles per DMA chunk (98 tiles = 7 chunks of 14)

# Matmul operand dtype: "f32" (exact, 2 HW passes), "f32r" (same fp32 bits,
# single-pass reduced-precision multiply), "bf16" (half DMA traffic too).
MM_DTYPE = os.environ.get("KERNEL_MM_DTYPE", "bf16")
# Output DMA dtype: bf16 halves the writeback traffic; host upcasts to f32.
OUT_BF16 = os.environ.get("KERNEL_OUT_BF16", "1") == "1"

_prog_cache = {}


def _np_mm_dtype():
    if MM_DTYPE == "bf16":
        import ml_dtypes
        return ml_dtypes.bfloat16
    return np.float32


def _np_out_dtype():
    if OUT_BF16:
        import ml_dtypes
        return ml_dtypes.bfloat16
    return np.float32


def _build_program(T, k_s):
    import concourse.bacc as bacc
    import concourse.tile as tile
    from concourse import mybir

    f32 = mybir.dt.float32
    mdt = {"f32": mybir.dt.float32, "f32r": mybir.dt.float32r,
           "bf16": mybir.dt.bfloat16}[MM_DTYPE]
    odt = mybir.dt.bfloat16 if OUT_BF16 else f32
    Relu = mybir.ActivationFunctionType.Relu
    Epad = T * TILE

    nc = bacc.Bacc("TRN2", target_bir_lowering=False, debug=False,
                   num_devices=N_CORES)
    inTd = nc.dram_tensor("inT", [P, 3 * Epad], mdt, kind="ExternalInput")
    w1d = nc.dram_tensor("w1", [P, 3 * P], mdt, kind="ExternalInput")
    w2d = nc.dram_tensor("w2", [P, P], mdt, kind="ExternalInput")
    b2d = nc.dram_tensor("b2c", [P, 1], f32, kind="ExternalInput")
    seld = nc.dram_tensor("sel", [k_s, Epad], mdt, kind="ExternalInput")
    zwd = nc.dram_tensor("zw", [k_s, T * P], mdt, kind="ExternalInput")
    outT = nc.dram_tensor("outT", [P, Epad], odt, kind="ExternalOutput")

    assert T % CH == 0
    n_chunks = T // CH
    CW = CH * TILE  # chunk width in edges

    with tile.TileContext(nc) as tc:
        with (
            tc.tile_pool(name="const", bufs=1) as constp,
            tc.tile_pool(name="inp", bufs=2) as inp,
            tc.tile_pool(name="selp", bufs=2) as selp,
            tc.tile_pool(name="hp", bufs=4) as hp,
            tc.tile_pool(name="outp", bufs=2) as outp,
            tc.tile_pool(name="psum", bufs=4, space="PSUM") as psum,
        ):
            w1s = constp.tile([P, 3 * P], mdt, tag="w1s", name="w1s")
            w2s = constp.tile([P, P], mdt, tag="w2s", name="w2s")
            b2s = constp.tile([P, 1], f32, tag="b2s", name="b2s")
            zws = constp.tile([k_s, T * P], mdt, tag="zws", name="zws")
            nc.gpsimd.dma_start(w1s[:], w1d[:])
            nc.gpsimd.dma_start(w2s[:], w2d[:])
            nc.gpsimd.dma_start(b2s[:], b2d[:])
            nc.gpsimd.dma_start(zws[:], zwd[:])

            for c in range(n_chunks):
                base = c * CW
                it = inp.tile([P, 3 * CW], mdt, tag="in", name=f"it{c}")
                nc.sync.dma_start(it[:], inTd[:, 3 * base:3 * base + 3 * CW])
                slt = selp.tile([k_s, CW], mdt, tag="sel", name=f"slt{c}")
                nc.gpsimd.dma_start(slt[:], seld[:, base:base + CW])
                ot = outp.tile([P, CW], odt, tag="o", name=f"ot{c}")

                for tl in range(CH):
                    t = c * CH + tl
                    cs = slice(tl * TILE, (tl + 1) * TILE)
                    p1 = psum.tile([P, TILE], f32, tag="p1", name=f"p1_{t}")
                    nc.tensor.matmul(p1[:], w1s[:, 0:P],
                                     it[:, tl * TILE:(tl + 1) * TILE],
                                     start=True, stop=False)
                    nc.tensor.matmul(p1[:], w1s[:, P:2 * P],
                                     it[:, CW + tl * TILE:CW + (tl + 1) * TILE],
                                     start=False, stop=False)
                    nc.tensor.matmul(p1[:], w1s[:, 2 * P:3 * P],
                                     it[:, 2 * CW + tl * TILE:2 * CW + (tl + 1) * TILE],
                                     start=False, stop=False)
                    for j0 in range(0, k_s, P):
                        j1 = min(j0 + P, k_s)
                        nc.tensor.matmul(p1[:],
                                         zws[j0:j1, t * P:(t + 1) * P],
                                         slt[j0:j1, cs],
                                         start=False, stop=(j1 == k_s))
                    h = hp.tile([P, TILE], mdt, tag="h", name=f"h{t}")
                    nc.scalar.activation(h[:], p1[:], Relu)
                    p2 = psum.tile([P, TILE], f32, tag="p2", name=f"p2_{t}")
                    nc.tensor.matmul(p2[:], w2s[:], h[:], start=True, stop=True)
                    nc.vector.tensor_scalar_add(ot[:, cs], p2[:], b2s[:])

                nc.scalar.dma_start(outT[:, base:base + CW], ot[:])

    nc.compile()
    return nc


def _get_program(T, k_s):
    key = (T, k_s)
    if key not in _prog_cache:
        _prog_cache[key] = _build_program(T, k_s)
    return _prog_cache[key]


def _install_profile_shim():
    """Optional: enable NTFF profiling under axon (KERNEL_PROFILE=1)."""
    import sys, types
    if "antenv.axon_hooks" not in sys.modules:
        mod = types.ModuleType("antenv.axon_hooks")
        mod._hook = None
        mod.set_axon_ntff_profile_hook = lambda h: setattr(mod, "_hook", h)
        mod.get_axon_ntff_profile_hook = lambda: mod._hook
        sys.modules["antenv.axon_hooks"] = mod
        try:
            import antenv
            antenv.axon_hooks = mod
        except ImportError:
            pass
        try:
            from trn_agent_boot.trn_boot import _ntff_profile_via_ctypes
            mod.set_axon_ntff_profile_hook(
                _ntff_profile_via_ctypes("/opt/axon/libaxon_pjrt.so"))
        except Exception:
            pass
    import concourse.bass_utils as bass_utils
    bass_utils.upload_artifacts = lambda tmpdir: tmpdir


def kernel(src, dest, edge_attr, u, batch, W1, b1, W2, b2):
    src = np.asarray(src, dtype=np.float32)
    dest = np.asarray(dest, dtype=np.float32)
    edge_attr = np.asarray(edge_attr, dtype=np.float32)
    u = np.asarray(u, dtype=np.float32)
    W1 = np.asarray(W1, dtype=np.float32)
    b1 = np.asarray(b1, dtype=np.float32)
    W2 = np.asarray(W2, dtype=np.float32)
    b2 = np.asarray(b2, dtype=np.float32)
    b = np.asarray(batch).astype(np.int64)

    E, D = src.shape
    G = u.shape[0]
    assert D == P and E % N_CORES == 0
    E0 = E // N_CORES
    CW = CH * TILE
    Epad = ((E0 + CW - 1) // CW) * CW
    T = Epad // TILE
    n_chunks = T // CH

    # Fold u[batch] @ W1d + b1 into a per-group table (tiny: G x D).
    z = (u @ W1[3 * D:4 * D] + b1).astype(np.float32)  # [G, D]

    # Per-core: tile-local group offsets for the z-selection matmul.
    g0s, js = [], []
    k_s = 1
    for c in range(N_CORES):
        bc = b[c * E0:(c + 1) * E0]
        bp = np.concatenate([bc, np.full(Epad - E0, bc[-1], dtype=np.int64)])
        per_tile = bp.reshape(T, TILE)
        g0 = per_tile.min(axis=1)                 # [T]
        j = bp - np.repeat(g0, TILE)              # [Epad], >= 0
        g0s.append(g0)
        js.append(j)
        k_s = max(k_s, int(j.max()) + 1)

    mmdt = _np_mm_dtype()
    src_m = src.astype(mmdt)
    dest_m = dest.astype(mmdt)
    ea_m = edge_attr.astype(mmdt)
    in_maps = []
    w1_in = np.ascontiguousarray(
        np.concatenate([W1[0:D], W1[D:2 * D], W1[2 * D:3 * D]], axis=1)
    ).astype(mmdt)                                # [D, 3D] = [W1a | W1b | W1c]
    w2_in = np.ascontiguousarray(W2).astype(mmdt)
    b2_in = np.ascontiguousarray(b2.reshape(P, 1))
    for c in range(N_CORES):
        sl = slice(c * E0, (c + 1) * E0)

        # Chunk-interleaved input: per chunk, [src | dest | ea] blocks of CW.
        inT = np.zeros((P, n_chunks, 3, CW), dtype=mmdt)
        for si, xm in enumerate((src_m, dest_m, ea_m)):
            xt = np.zeros((P, Epad), dtype=mmdt)
            xt[:, :E0] = xm[sl].T
            inT[:, :, si, :] = xt.reshape(P, n_chunks, CW)

        selc = np.zeros((k_s, Epad), dtype=mmdt)
        selc[js[c], np.arange(Epad)] = 1.0
        selc[:, E0:] = 0.0  # pad edges contribute nothing
        gidx = np.clip(g0s[c][:, None] + np.arange(k_s)[None, :], 0, G - 1)
        zwc = np.ascontiguousarray(
            z[gidx].transpose(1, 0, 2).reshape(k_s, T * P)).astype(mmdt)
        in_maps.append({
            "inT": inT.reshape(P, 3 * Epad),
            "w1": w1_in, "w2": w2_in, "b2c": b2_in,
            "sel": selc, "zw": zwc,
        })

    profile = os.environ.get("KERNEL_PROFILE", "") == "1"
    if profile:
        _install_profile_shim()

    nc = _get_program(T, k_s)
    from concourse.bass_utils import run_bass_kernel_spmd
    kwargs = {}
    if profile:
        kwargs["trace"] = True
        if os.environ.get("KERNEL_PROFILE_ALL", "") == "1":
            kwargs["trace_cores"] = list(range(N_CORES))
    res = run_bass_kernel_spmd(nc, in_maps, core_ids=list(range(N_CORES)),
                               **kwargs)
    if profile and res.exec_time_ns is not None:
        with open("/tmp/kernel_exec_ns.txt", "w") as f:
            f.write(str(res.exec_time_ns))
        print(f"HW exec time: {res.exec_time_ns} ns")

    out = np.empty((E, P), dtype=np.float32)
    for c in range(N_CORES):
        out[c * E0:(c + 1) * E0] = \
            res.results[c]["outT"][:, :E0].T.astype(np.float32)
    return out


# revision 3
# speedup vs baseline: 1.3273x; 1.1052x over previous
# Trainium2 Bass kernel for the MEGNet edge model:
#   out = relu(concat([src, dest, edge_attr, u[batch]], 1) @ W1 + b1) @ W2 + b2
#
# Strategy (8 NeuronCores, SPMD, edges sharded contiguously):
#  * All tensors are shipped to the device in a transposed, feature-major
#    layout [128, E_pad] so the PE array can contract over features without
#    any on-chip transposes; the host transposes shards and transposes the
#    output back.
#  * The three big input streams are sent in bf16 (measured rel err ~4e-3
#    vs a 2e-2 gate) and the output is DMAed back as bf16 and upcast to
#    fp32 on the host.  This halves HBM traffic; the kernel is memory-bound.
#  * Edges are processed in subgroups of 4 matmul tiles (4x512 edges).  The
#    input streams are interleaved subgroup-wise in DRAM as [src|dest|ea]
#    blocks so each subgroup needs ONE contiguous ~1.6MB DMA; the output is
#    written back per subgroup (~0.5MB).  Fine granularity keeps the DMA
#    queues busy end-to-end and shrinks pipeline ramp-in/ramp-out.
#  * comb @ W1 decomposes into src@W1a + dest@W1b + edge_attr@W1c +
#    u[batch]@W1d.  The u[batch] term plus b1 is folded into a per-group
#    table z = u @ W1d + b1 [G, 128]; since batch is sorted, each 512-edge
#    tile only spans a few consecutive groups, so z[batch] is applied with
#    one extra small matmul per tile (one-hot selection built on the host).
#  * Within a subgroup, matmuls are ordered weight-stationary (w1a over all
#    tiles, then w1b, ...) to minimize LDWEIGHTS churn.  PSUM accumulates
#    the 4 terms; ScalarE applies ReLU (PSUM->SBUF); the second matmul uses
#    W2; VectorE adds b2 and downcasts to bf16.
#  * The PE clock is HAM-gated (1.2 GHz until ~3.4us of sustained activity).
#    A run of dummy matmuls on scratch SBUF warms the array to 2.4 GHz
#    while the first input DMA is still in flight.
import os
import numpy as np

N_CORES = 8
P = 128      # feature dim == SBUF partitions
TILE = 512   # edges per matmul tile (one PSUM bank of fp32)
SG = 4       # tiles per subgroup (one DMA + one PSUM wave)

MM_DTYPE = os.environ.get("KERNEL_MM_DTYPE", "bf16")
OUT_BF16 = os.environ.get("KERNEL_OUT_BF16", "1") == "1"
N_WARM = int(os.environ.get("KERNEL_WARMUP_MM", "46"))

_prog_cache = {}


def _np_mm_dtype():
    if MM_DTYPE == "bf16":
        import ml_dtypes
        return ml_dtypes.bfloat16
    return np.float32


def _schedule(T):
    """Subgroups of SG tiles (last one ragged)."""
    sched = []
    t = 0
    while t < T:
        n = min(SG, T - t)
        sched.append((t, n))
        t += n
    return sched


def _build_program(T, k_s):
    import concourse.bacc as bacc
    import concourse.tile as tile
    from concourse import mybir

    f32 = mybir.dt.float32
    mdt = {"f32": mybir.dt.float32, "f32r": mybir.dt.float32r,
           "bf16": mybir.dt.bfloat16}[MM_DTYPE]
    odt = mybir.dt.bfloat16 if OUT_BF16 else f32
    Relu = mybir.ActivationFunctionType.Relu
    Epad = T * TILE

    nc = bacc.Bacc("TRN2", target_bir_lowering=False, debug=False,
                   num_devices=N_CORES)
    inTd = nc.dram_tensor("inT", [P, 3 * Epad], mdt, kind="ExternalInput")
    w1d = nc.dram_tensor("w1", [P, 3 * P], mdt, kind="ExternalInput")
    w2d = nc.dram_tensor("w2", [P, P], mdt, kind="ExternalInput")
    b2d = nc.dram_tensor("b2c", [P, 1], f32, kind="ExternalInput")
    seld = nc.dram_tensor("sel", [k_s, Epad], mdt, kind="ExternalInput")
    zwd = nc.dram_tensor("zw", [k_s, T * P], mdt, kind="ExternalInput")
    outT = nc.dram_tensor("outT", [P, Epad], odt, kind="ExternalOutput")

    sched = _schedule(T)

    with tile.TileContext(nc) as tc:
        with (
            tc.tile_pool(name="const", bufs=1) as constp,
            tc.tile_pool(name="inp", bufs=3) as inp,
            tc.tile_pool(name="hp", bufs=8) as hp,
            tc.tile_pool(name="outp", bufs=3) as outp,
            tc.tile_pool(name="ps1", bufs=4, space="PSUM") as ps1,
            tc.tile_pool(name="ps2", bufs=3, space="PSUM") as ps2,
            tc.tile_pool(name="psw", bufs=1, space="PSUM") as psw,
        ):
            # --- PE warm-up: dummy matmuls on scratch SBUF while the first
            # input DMA is in flight (HAM releases the clock gate after
            # ~3.4us of sustained tensor activity).
            scr = constp.tile([P, TILE], mdt, tag="scr", name="scr")
            nc.vector.memset(scr[:], 0.0)
            pw = psw.tile([P, TILE], f32, tag="pw", name="pw")
            for i in range(N_WARM):
                nc.tensor.matmul(pw[:], scr[:, 0:P], scr[:],
                                 start=True, stop=True)

            # --- constants
            w1s = constp.tile([P, 3 * P], mdt, tag="w1s", name="w1s")
            w2s = constp.tile([P, P], mdt, tag="w2s", name="w2s")
            b2s = constp.tile([P, 1], f32, tag="b2s", name="b2s")
            zws = constp.tile([k_s, T * P], mdt, tag="zws", name="zws")
            sels = constp.tile([k_s, Epad], mdt, tag="sels", name="sels")
            nc.gpsimd.dma_start(w1s[:], w1d[:])
            nc.gpsimd.dma_start(w2s[:], w2d[:])
            nc.gpsimd.dma_start(b2s[:], b2d[:])
            nc.gpsimd.dma_start(zws[:], zwd[:])
            nc.gpsimd.dma_start(sels[:], seld[:])

            for gi, (t0, n) in enumerate(sched):
                cw = n * TILE
                base = t0 * TILE
                tag_sfx = "" if n == SG else f"_{n}"
                it = inp.tile([P, 3 * cw], mdt, tag="in" + tag_sfx,
                              name=f"it{gi}")
                nc.sync.dma_start(it[:], inTd[:, 3 * base:3 * base + 3 * cw])
                ot = outp.tile([P, cw], odt, tag="o" + tag_sfx,
                               name=f"ot{gi}")

                p1s = [ps1.tile([P, TILE], f32, tag="p1", name=f"p1_{t0}_{i}")
                       for i in range(n)]
                # weight-stationary sweeps: w1a, w1b, w1c across the subgroup
                for s in range(3):
                    for i in range(n):
                        nc.tensor.matmul(
                            p1s[i][:], w1s[:, s * P:(s + 1) * P],
                            it[:, s * cw + i * TILE:s * cw + (i + 1) * TILE],
                            start=(s == 0), stop=False)
                # per-tile z-selection matmul closes the accumulation
                for i in range(n):
                    t = t0 + i
                    for j0 in range(0, k_s, P):
                        j1 = min(j0 + P, k_s)
                        nc.tensor.matmul(
                            p1s[i][:], zws[j0:j1, t * P:(t + 1) * P],
                            sels[j0:j1, base + i * TILE:base + (i + 1) * TILE],
                            start=False, stop=(j1 == k_s))
                hs = [hp.tile([P, TILE], mdt, tag="h", name=f"h{t0}_{i}")
                      for i in range(n)]
                for i in range(n):
                    nc.scalar.activation(hs[i][:], p1s[i][:], Relu)
                p2s = [ps2.tile([P, TILE], f32, tag="p2", name=f"p2_{t0}_{i}")
                       for i in range(n)]
                for i in range(n):
                    nc.tensor.matmul(p2s[i][:], w2s[:], hs[i][:],
                                     start=True, stop=True)
                for i in range(n):
                    nc.vector.tensor_scalar_add(
                        ot[:, i * TILE:(i + 1) * TILE], p2s[i][:], b2s[:])

                nc.scalar.dma_start(outT[:, base:base + cw], ot[:])

    nc.compile()
    return nc


def _get_program(T, k_s):
    key = (T, k_s)
    if key not in _prog_cache:
        _prog_cache[key] = _build_program(T, k_s)
    return _prog_cache[key]


def _install_profile_shim():
    """Optional: enable NTFF profiling under axon (KERNEL_PROFILE=1)."""
    import sys, types
    if "antenv.axon_hooks" not in sys.modules:
        mod = types.ModuleType("antenv.axon_hooks")
        mod._hook = None
        mod.set_axon_ntff_profile_hook = lambda h: setattr(mod, "_hook", h)
        mod.get_axon_ntff_profile_hook = lambda: mod._hook
        sys.modules["antenv.axon_hooks"] = mod
        try:
            import antenv
            antenv.axon_hooks = mod
        except ImportError:
            pass
        try:
            from trn_agent_boot.trn_boot import _ntff_profile_via_ctypes
            mod.set_axon_ntff_profile_hook(
                _ntff_profile_via_ctypes("/opt/axon/libaxon_pjrt.so"))
        except Exception:
            pass
    import concourse.bass_utils as bass_utils
    bass_utils.upload_artifacts = lambda tmpdir: tmpdir


def kernel(src, dest, edge_attr, u, batch, W1, b1, W2, b2):
    src = np.asarray(src, dtype=np.float32)
    dest = np.asarray(dest, dtype=np.float32)
    edge_attr = np.asarray(edge_attr, dtype=np.float32)
    u = np.asarray(u, dtype=np.float32)
    W1 = np.asarray(W1, dtype=np.float32)
    b1 = np.asarray(b1, dtype=np.float32)
    W2 = np.asarray(W2, dtype=np.float32)
    b2 = np.asarray(b2, dtype=np.float32)
    b = np.asarray(batch).astype(np.int64)

    E, D = src.shape
    G = u.shape[0]
    assert D == P and E % N_CORES == 0
    E0 = E // N_CORES
    Epad = ((E0 + TILE - 1) // TILE) * TILE
    T = Epad // TILE

    # Fold u[batch] @ W1d + b1 into a per-group table (tiny: G x D).
    z = (u @ W1[3 * D:4 * D] + b1).astype(np.float32)  # [G, D]

    # Per-core: tile-local group offsets for the z-selection matmul.
    g0s, js = [], []
    k_s = 1
    for c in range(N_CORES):
        bc = b[c * E0:(c + 1) * E0]
        bp = np.concatenate([bc, np.full(Epad - E0, bc[-1], dtype=np.int64)])
        per_tile = bp.reshape(T, TILE)
        g0 = per_tile.min(axis=1)                 # [T]
        j = bp - np.repeat(g0, TILE)              # [Epad], >= 0
        g0s.append(g0)
        js.append(j)
        k_s = max(k_s, int(j.max()) + 1)

    sched = _schedule(T)
    mmdt = _np_mm_dtype()
    src_m = src.astype(mmdt)
    dest_m = dest.astype(mmdt)
    ea_m = edge_attr.astype(mmdt)
    in_maps = []
    w1_in = np.ascontiguousarray(
        np.concatenate([W1[0:D], W1[D:2 * D], W1[2 * D:3 * D]], axis=1)
    ).astype(mmdt)                                # [D, 3D] = [W1a | W1b | W1c]
    w2_in = np.ascontiguousarray(W2).astype(mmdt)
    b2_in = np.ascontiguousarray(b2.reshape(P, 1))
    for c in range(N_CORES):
        sl = slice(c * E0, (c + 1) * E0)

        # Subgroup-interleaved input: per subgroup, [src|dest|ea] blocks.
        streams = []
        for xm in (src_m, dest_m, ea_m):
            xt = np.zeros((P, Epad), dtype=mmdt)
            xt[:, :E0] = xm[sl].T
            streams.append(xt)
        inT = np.empty((P, 3 * Epad), dtype=mmdt)
        for (t0, n) in sched:
            cw = n * TILE
            base = t0 * TILE
            for si, xt in enumerate(streams):
                inT[:, 3 * base + si * cw:3 * base + (si + 1) * cw] = \
                    xt[:, base:base + cw]

        selc = np.zeros((k_s, Epad), dtype=mmdt)
        selc[js[c], np.arange(Epad)] = 1.0
        selc[:, E0:] = 0.0  # pad edges contribute nothing
        gidx = np.clip(g0s[c][:, None] + np.arange(k_s)[None, :], 0, G - 1)
        zwc = np.ascontiguousarray(
            z[gidx].transpose(1, 0, 2).reshape(k_s, T * P)).astype(mmdt)
        in_maps.append({
            "inT": inT,
            "w1": w1_in, "w2": w2_in, "b2c": b2_in,
            "sel": selc, "zw": zwc,
        })

    profile = os.environ.get("KERNEL_PROFILE", "") == "1"
    if profile:
        _install_profile_shim()

    nc = _get_program(T, k_s)
    from concourse.bass_utils import run_bass_kernel_spmd
    kwargs = {}
    if profile:
        kwargs["trace"] = True
        if os.environ.get("KERNEL_PROFILE_ALL", "") == "1":
            kwargs["trace_cores"] = list(range(N_CORES))
    res = run_bass_kernel_spmd(nc, in_maps, core_ids=list(range(N_CORES)),
                               **kwargs)
    if profile and res.exec_time_ns is not None:
        with open("/tmp/kernel_exec_ns.txt", "w") as f:
            f.write(str(res.exec_time_ns))
        print(f"HW exec time: {res.exec_time_ns} ns")

    out = np.empty((E, P), dtype=np.float32)
    for c in range(N_CORES):
        out[c * E0:(c + 1) * E0] = \
            res.results[c]["outT"][:, :E0].T.astype(np.float32)
    return out


# revision 7
# speedup vs baseline: 1.3350x; 1.0058x over previous
# Trainium2 Bass kernel for the MEGNet edge model:
#   out = relu(concat([src, dest, edge_attr, u[batch]], 1) @ W1 + b1) @ W2 + b2
#
# Strategy (8 NeuronCores, SPMD, edges sharded contiguously):
#  * All tensors are shipped to the device in a transposed, feature-major
#    layout [128, E_pad] so the PE array can contract over features without
#    any on-chip transposes; the host transposes shards and transposes the
#    output back.
#  * The three big input streams are sent in bf16 (measured rel err ~4e-3
#    vs a 2e-2 gate) and the output is DMAed back as bf16 and upcast to
#    fp32 on the host.  This halves HBM traffic; the kernel is memory-bound.
#  * Edges are processed in subgroups of 4 matmul tiles (4x512 edges).  The
#    input streams are interleaved subgroup-wise in DRAM as [src|dest|ea]
#    blocks so each subgroup needs ONE contiguous ~1.6MB DMA; the output is
#    written back per subgroup (~0.5MB).  Fine granularity keeps the DMA
#    queues busy end-to-end and shrinks pipeline ramp-in/ramp-out.
#  * comb @ W1 decomposes into src@W1a + dest@W1b + edge_attr@W1c +
#    u[batch]@W1d.  The u[batch] term plus b1 is folded into a per-group
#    table z = u @ W1d + b1 [G, 128]; since batch is sorted, each 512-edge
#    tile only spans a few consecutive groups, so z[batch] is applied with
#    one extra small matmul per tile (one-hot selection built on the host).
#  * Within a subgroup, matmuls are ordered weight-stationary (w1a over all
#    tiles, then w1b, ...) to minimize LDWEIGHTS churn.  PSUM accumulates
#    the 4 terms; ScalarE applies ReLU (PSUM->SBUF); the second matmul uses
#    W2; VectorE adds b2 and downcasts to bf16.
#  * The PE clock is HAM-gated (1.2 GHz until ~3.4us of sustained activity).
#    A run of dummy matmuls on scratch SBUF warms the array to 2.4 GHz
#    while the first input DMA is still in flight.
import os
import numpy as np

N_CORES = 8
P = 128      # feature dim == SBUF partitions
TILE = 512   # edges per matmul tile (one PSUM bank of fp32)
SG = 4       # tiles per subgroup (one DMA + one PSUM wave)

MM_DTYPE = os.environ.get("KERNEL_MM_DTYPE", "bf16")
OUT_BF16 = os.environ.get("KERNEL_OUT_BF16", "1") == "1"
N_WARM = int(os.environ.get("KERNEL_WARMUP_MM", "20"))
FILL_SG = int(os.environ.get("KERNEL_FILL_PER_SG", "6"))

_prog_cache = {}


def _np_mm_dtype():
    if MM_DTYPE == "bf16":
        import ml_dtypes
        return ml_dtypes.bfloat16
    return np.float32


def _schedule(T):
    """Subgroups of SG tiles (last one ragged)."""
    sched = []
    t = 0
    while t < T:
        n = min(SG, T - t)
        sched.append((t, n))
        t += n
    return sched


def _build_program(T, k_s):
    import concourse.bacc as bacc
    import concourse.tile as tile
    from concourse import mybir

    f32 = mybir.dt.float32
    mdt = {"f32": mybir.dt.float32, "f32r": mybir.dt.float32r,
           "bf16": mybir.dt.bfloat16}[MM_DTYPE]
    odt = mybir.dt.bfloat16 if OUT_BF16 else f32
    Relu = mybir.ActivationFunctionType.Relu
    Epad = T * TILE

    nc = bacc.Bacc("TRN2", target_bir_lowering=False, debug=False,
                   num_devices=N_CORES)
    inTd = nc.dram_tensor("inT", [P, 3 * Epad], mdt, kind="ExternalInput")
    w1d = nc.dram_tensor("w1", [P, 3 * P], mdt, kind="ExternalInput")
    w2d = nc.dram_tensor("w2", [P, P], mdt, kind="ExternalInput")
    b2d = nc.dram_tensor("b2c", [P, 1], f32, kind="ExternalInput")
    seld = nc.dram_tensor("sel", [k_s, Epad], mdt, kind="ExternalInput")
    zwd = nc.dram_tensor("zw", [k_s, T * P], mdt, kind="ExternalInput")
    outT = nc.dram_tensor("outT", [P, Epad], odt, kind="ExternalOutput")

    sched = _schedule(T)

    with tile.TileContext(nc) as tc:
        with (
            tc.tile_pool(name="const", bufs=1) as constp,
            tc.tile_pool(name="inp", bufs=3) as inp,
            tc.tile_pool(name="hp", bufs=8) as hp,
            tc.tile_pool(name="outp", bufs=3) as outp,
            tc.tile_pool(name="ps1", bufs=4, space="PSUM") as ps1,
            tc.tile_pool(name="ps2", bufs=3, space="PSUM") as ps2,
            tc.tile_pool(name="psw", bufs=1, space="PSUM") as psw,
        ):
            # --- PE warm-up: dummy matmuls on scratch SBUF while the first
            # input DMA is in flight (HAM releases the clock gate after
            # ~3.4us of sustained tensor activity).
            scr = constp.tile([P, TILE], mdt, tag="scr", name="scr")
            nc.vector.memset(scr[:], 0.0)
            pw = psw.tile([P, TILE], f32, tag="pw", name="pw")
            for i in range(N_WARM):
                nc.tensor.matmul(pw[:], scr[:, 0:P], scr[:],
                                 start=True, stop=True)

            # --- constants (all small; zws is only k_s partitions wide)
            w1s = constp.tile([P, 3 * P], mdt, tag="w1s", name="w1s")
            w2s = constp.tile([P, P], mdt, tag="w2s", name="w2s")
            b2s = constp.tile([P, 1], f32, tag="b2s", name="b2s")
            zws = constp.tile([k_s, T * P], mdt, tag="zws", name="zws")
            nc.gpsimd.dma_start(w1s[:], w1d[:])
            nc.gpsimd.dma_start(zws[:], zwd[:])
            nc.gpsimd.dma_start(w2s[:], w2d[:])
            nc.gpsimd.dma_start(b2s[:], b2d[:])

            for gi, (t0, n) in enumerate(sched):
                cw = n * TILE
                base = t0 * TILE
                tag_sfx = "" if n == SG else f"_{n}"
                it = inp.tile([P, 3 * cw], mdt, tag="in" + tag_sfx,
                              name=f"it{gi}")
                nc.sync.dma_start(it[:], inTd[:, 3 * base:3 * base + 3 * cw])
                # sel rows for this subgroup: tiny, streamed on the scalar
                # HWDGE queue so it never blocks the big input stream
                sels = inp.tile([k_s, cw], mdt, tag="sel" + tag_sfx,
                                name=f"sel{gi}")
                nc.scalar.dma_start(sels[:], seld[:, base:base + cw])
                ot = outp.tile([P, cw], odt, tag="o" + tag_sfx,
                               name=f"ot{gi}")

                p1s = [ps1.tile([P, TILE], f32, tag="p1", name=f"p1_{t0}_{i}")
                       for i in range(n)]
                # weight-stationary sweeps: w1a, w1b, w1c across the subgroup
                for s in range(3):
                    for i in range(n):
                        nc.tensor.matmul(
                            p1s[i][:], w1s[:, s * P:(s + 1) * P],
                            it[:, s * cw + i * TILE:s * cw + (i + 1) * TILE],
                            start=(s == 0), stop=False)
                # per-tile z-selection matmul closes the accumulation
                for i in range(n):
                    t = t0 + i
                    for j0 in range(0, k_s, P):
                        j1 = min(j0 + P, k_s)
                        nc.tensor.matmul(
                            p1s[i][:], zws[j0:j1, t * P:(t + 1) * P],
                            sels[j0:j1, i * TILE:(i + 1) * TILE],
                            start=False, stop=(j1 == k_s))
                hs = [hp.tile([P, TILE], mdt, tag="h", name=f"h{t0}_{i}")
                      for i in range(n)]
                for i in range(n):
                    nc.scalar.activation(hs[i][:], p1s[i][:], Relu)
                p2s = [ps2.tile([P, TILE], f32, tag="p2", name=f"p2_{t0}_{i}")
                       for i in range(n)]
                for i in range(n):
                    nc.tensor.matmul(p2s[i][:], w2s[:], hs[i][:],
                                     start=True, stop=True)
                # filler matmuls on scratch keep the PE duty cycle high so
                # the HAM clock gate never drops back to 1.2 GHz while the
                # (memory-bound) pipeline waits on the next input DMA
                if gi < len(sched) - 2:
                    for i in range(FILL_SG):
                        nc.tensor.matmul(pw[:], scr[:, 0:P], scr[:],
                                         start=True, stop=True)
                for i in range(n):
                    nc.vector.tensor_scalar_add(
                        ot[:, i * TILE:(i + 1) * TILE], p2s[i][:], b2s[:])

                nc.scalar.dma_start(outT[:, base:base + cw], ot[:])

    nc.compile()
    return nc


def _get_program(T, k_s):
    key = (T, k_s)
    if key not in _prog_cache:
        _prog_cache[key] = _build_program(T, k_s)
    return _prog_cache[key]


def _install_profile_shim():
    """Optional: enable NTFF profiling under axon (KERNEL_PROFILE=1)."""
    import sys, types
    if "antenv.axon_hooks" not in sys.modules:
        mod = types.ModuleType("antenv.axon_hooks")
        mod._hook = None
        mod.set_axon_ntff_profile_hook = lambda h: setattr(mod, "_hook", h)
        mod.get_axon_ntff_profile_hook = lambda: mod._hook
        sys.modules["antenv.axon_hooks"] = mod
        try:
            import antenv
            antenv.axon_hooks = mod
        except ImportError:
            pass
        try:
            from trn_agent_boot.trn_boot import _ntff_profile_via_ctypes
            mod.set_axon_ntff_profile_hook(
                _ntff_profile_via_ctypes("/opt/axon/libaxon_pjrt.so"))
        except Exception:
            pass
    import concourse.bass_utils as bass_utils
    bass_utils.upload_artifacts = lambda tmpdir: tmpdir


def kernel(src, dest, edge_attr, u, batch, W1, b1, W2, b2):
    src = np.asarray(src, dtype=np.float32)
    dest = np.asarray(dest, dtype=np.float32)
    edge_attr = np.asarray(edge_attr, dtype=np.float32)
    u = np.asarray(u, dtype=np.float32)
    W1 = np.asarray(W1, dtype=np.float32)
    b1 = np.asarray(b1, dtype=np.float32)
    W2 = np.asarray(W2, dtype=np.float32)
    b2 = np.asarray(b2, dtype=np.float32)
    b = np.asarray(batch).astype(np.int64)

    E, D = src.shape
    G = u.shape[0]
    assert D == P and E % N_CORES == 0
    E0 = E // N_CORES
    Epad = ((E0 + TILE - 1) // TILE) * TILE
    T = Epad // TILE

    # Fold u[batch] @ W1d + b1 into a per-group table (tiny: G x D).
    z = (u @ W1[3 * D:4 * D] + b1).astype(np.float32)  # [G, D]

    # Per-core: tile-local group offsets for the z-selection matmul.
    g0s, js = [], []
    k_s = 1
    for c in range(N_CORES):
        bc = b[c * E0:(c + 1) * E0]
        bp = np.concatenate([bc, np.full(Epad - E0, bc[-1], dtype=np.int64)])
        per_tile = bp.reshape(T, TILE)
        g0 = per_tile.min(axis=1)                 # [T]
        j = bp - np.repeat(g0, TILE)              # [Epad], >= 0
        g0s.append(g0)
        js.append(j)
        k_s = max(k_s, int(j.max()) + 1)

    sched = _schedule(T)
    mmdt = _np_mm_dtype()
    src_m = src.astype(mmdt)
    dest_m = dest.astype(mmdt)
    ea_m = edge_attr.astype(mmdt)
    in_maps = []
    w1_in = np.ascontiguousarray(
        np.concatenate([W1[0:D], W1[D:2 * D], W1[2 * D:3 * D]], axis=1)
    ).astype(mmdt)                                # [D, 3D] = [W1a | W1b | W1c]
    w2_in = np.ascontiguousarray(W2).astype(mmdt)
    b2_in = np.ascontiguousarray(b2.reshape(P, 1))
    for c in range(N_CORES):
        sl = slice(c * E0, (c + 1) * E0)

        # Subgroup-interleaved input: per subgroup, [src|dest|ea] blocks.
        streams = []
        for xm in (src_m, dest_m, ea_m):
            xt = np.zeros((P, Epad), dtype=mmdt)
            xt[:, :E0] = xm[sl].T
            streams.append(xt)
        inT = np.empty((P, 3 * Epad), dtype=mmdt)
        for (t0, n) in sched:
            cw = n * TILE
            base = t0 * TILE
            for si, xt in enumerate(streams):
                inT[:, 3 * base + si * cw:3 * base + (si + 1) * cw] = \
                    xt[:, base:base + cw]

        selc = np.zeros((k_s, Epad), dtype=mmdt)
        selc[js[c], np.arange(Epad)] = 1.0
        selc[:, E0:] = 0.0  # pad edges contribute nothing
        gidx = np.clip(g0s[c][:, None] + np.arange(k_s)[None, :], 0, G - 1)
        zwc = np.ascontiguousarray(
            z[gidx].transpose(1, 0, 2).reshape(k_s, T * P)).astype(mmdt)
        in_maps.append({
            "inT": inT,
            "w1": w1_in, "w2": w2_in, "b2c": b2_in,
            "sel": selc, "zw": zwc,
        })

    profile = os.environ.get("KERNEL_PROFILE", "") == "1"
    if profile:
        _install_profile_shim()

    nc = _get_program(T, k_s)
    from concourse.bass_utils import run_bass_kernel_spmd
    kwargs = {}
    if profile:
        kwargs["trace"] = True
        if os.environ.get("KERNEL_PROFILE_ALL", "") == "1":
            kwargs["trace_cores"] = list(range(N_CORES))
    res = run_bass_kernel_spmd(nc, in_maps, core_ids=list(range(N_CORES)),
                               **kwargs)
    if profile and res.exec_time_ns is not None:
        with open("/tmp/kernel_exec_ns.txt", "w") as f:
            f.write(str(res.exec_time_ns))
        print(f"HW exec time: {res.exec_time_ns} ns")

    out = np.empty((E, P), dtype=np.float32)
    for c in range(N_CORES):
        out[c * E0:(c + 1) * E0] = \
            res.results[c]["outT"][:, :E0].T.astype(np.float32)
    return out
